# revision 5
# baseline (speedup 1.0000x reference)
"""Trainium2 Bass kernel for nn_ExemplarNoAttention (retrieval_knn).

logits[b,c] = log(eps + sum_{e: label[e]==c} exp(-beta * ||x_b - E_e||^2))

Sharding: exemplar bank Ne=50000 split across 8 NeuronCores (1 SPMD program,
per-core exemplar slabs). Host does O(Ne*d) layout prep only: reorder
exemplars so each core's slab is grouped by class, with per-class segment
sizes identical on every core (padded slots get weight 0).

Factorization (keeps the GEMM at K=64 so back-to-back matmuls never reload
weights mid-batch-tile; weight loads for the next batch tile overlap in the
other 64 PE rows):
    exp(-beta*d2) = exp(2*beta*<x,e> - beta*x2 - C) * exp(-beta*e2) * e^C
Device per core:
  TensorE : psum[b,e] = 2*beta*<x_b, E_e>          (bf16, K=64)
  ScalarE : sims[b,e] = Exp(psum + (-beta*x2_b - C))   -> bf16 SBUF
  VectorE : class_sums[b,c] = sum_seg sims * w     (w = exp(-beta*e2), bf16)
            via affine_mul_reduce (fused multiply + accumulate)
  AllReduce per batch-tile of (128,10) partial sums across 8 cores
  ScalarE : logits = Ln(e^C * class_sums + eps) ; identical on all cores
"""

import os
import numpy as np
import ml_dtypes

NUM_CLASSES = 10
EPS = 1e-12
N_CORES = 8
B = 1024
D = 64
NE = 50000
BT = 128            # batch rows per psum tile (output partitions)
NBT = B // BT       # 8 batch tiles
SEG_ALIGN = 32      # per-class segment padding granularity
CHUNK = 512         # matmul moving-operand chunk (one PSUM bank)
WIN = 2048          # psum window drained by one activation (4 banks)
CSHIFT = 40.0       # exponent shift: sims stay in bf16 range
N_GPSIMD_CLASSES = 0  # GpSimd cannot run TensorScalarPtr (walrus engine check)

LAST_EXEC_NS = None
LAST_RESULTS = None
TRACE = bool(int(os.environ.get("KERNEL_TRACE", "0")))
TRACE_DIR = os.environ.get("KERNEL_TRACE_DIR", "")


def _host_prep(x, exemplars, exemplar_labels, beta_raw):
    x = np.asarray(x, dtype=np.float32)
    E = np.asarray(exemplars, dtype=np.float32)
    labels = np.asarray(exemplar_labels).astype(np.int64)
    beta = float(np.logaddexp(0.0, np.float64(beta_raw.reshape(-1)[0])))

    per_core_idx = [[None] * NUM_CLASSES for _ in range(N_CORES)]
    max_cc = np.zeros(NUM_CLASSES, dtype=np.int64)
    for c in range(NUM_CLASSES):
        idx_c = np.nonzero(labels == c)[0]
        n = len(idx_c)
        base, rem = divmod(n, N_CORES)
        sizes = [base + (1 if i < rem else 0) for i in range(N_CORES)]
        off = 0
        for i in range(N_CORES):
            per_core_idx[i][c] = idx_c[off:off + sizes[i]]
            off += sizes[i]
        max_cc[c] = max(sizes) if n else 0

    seg_sizes = [int(-(-m // SEG_ALIGN) * SEG_ALIGN) for m in max_cc]
    seg_offs = np.concatenate([[0], np.cumsum(seg_sizes)]).astype(np.int64)
    e_used = int(seg_offs[-1])
    e_pad = int(-(-e_used // CHUNK) * CHUNK)

    # per-core transposed exemplar slabs, duplicated into both 64-row halves
    # of the partition dim so even/odd batch tiles can use disjoint PE rows
    e2 = (E.astype(np.float64) ** 2).sum(axis=1)
    ea_cores = []
    w_cores = []
    for i in range(N_CORES):
        ea = np.zeros((D, e_pad), dtype=np.float32)
        w = np.zeros((e_pad,), dtype=np.float64)
        for c in range(NUM_CLASSES):
            idx = per_core_idx[i][c]
            o = int(seg_offs[c])
            if len(idx):
                ea[:, o:o + len(idx)] = (2.0 * beta) * E[idx].T
                w[o:o + len(idx)] = np.exp(-beta * e2[idx])
        ea2 = np.concatenate([ea, ea], axis=0).astype(ml_dtypes.bfloat16)
        ea_cores.append(ea2)
        wt = np.broadcast_to(
            w.astype(ml_dtypes.bfloat16)[None, :], (BT, e_pad)
        ).copy()
        w_cores.append(wt)

    xa = np.concatenate([x.T, x.T], axis=0).astype(ml_dtypes.bfloat16)  # (128,B)

    x2 = (x.astype(np.float64) ** 2).sum(axis=1)
    bias = (-beta * x2 - CSHIFT).astype(np.float32).reshape(NBT, BT).T.copy()

    return ea_cores, w_cores, xa, bias, seg_offs, seg_sizes, e_pad


def _build_program(seg_offs, seg_sizes, e_pad):
    from contextlib import ExitStack
    import concourse.bass as bass
    import concourse.tile as tile
    from concourse import bacc, mybir

    f32 = mybir.dt.float32
    bf16 = mybir.dt.bfloat16

    nc = bacc.Bacc(
        "TRN2",
        target_bir_lowering=False,
        debug=False,
        enable_asserts=False,
        num_devices=N_CORES,
    )

    ea_d = nc.dram_tensor("ea", [2 * D, e_pad], bf16, kind="ExternalInput").ap()
    w_d = nc.dram_tensor("w", [BT, e_pad], bf16, kind="ExternalInput").ap()
    xa_d = nc.dram_tensor("xa", [2 * D, B], bf16, kind="ExternalInput").ap()
    bias_d = nc.dram_tensor("biasx", [BT, NBT], f32, kind="ExternalInput").ap()
    out_d = nc.dram_tensor("logits", [B, NUM_CLASSES], f32, kind="ExternalOutput").ap()

    max_seg = max(seg_sizes)
    wins = []
    o = 0
    while o < e_pad:
        wins.append((o, min(WIN, e_pad - o)))
        o += WIN

    with tile.TileContext(nc) as tc, ExitStack() as ctx:
        const_pool = ctx.enter_context(tc.tile_pool(name="const", bufs=1))
        psum_pool = ctx.enter_context(tc.tile_pool(name="psum", bufs=2, space="PSUM"))
        sims_pool = ctx.enter_context(tc.tile_pool(name="sims", bufs=2))
        cls_pool = ctx.enter_context(tc.tile_pool(name="cls", bufs=3))
        junk_pool = ctx.enter_context(tc.tile_pool(name="junk", bufs=2))
        res_pool = ctx.enter_context(tc.tile_pool(name="res", bufs=1))
        dram_pool = ctx.enter_context(tc.tile_pool(name="dram", bufs=1, space="DRAM"))

        ea_t = const_pool.tile([2 * D, e_pad], bf16, name="ea_t")
        w_t = const_pool.tile([BT, e_pad], bf16, name="w_t")
        step = -(-(-(-e_pad // 8)) // CHUNK) * CHUNK
        o = 0
        while o < e_pad:
            wd = min(step, e_pad - o)
            nc.sync.dma_start(out=ea_t[:, o:o + wd], in_=ea_d[:, o:o + wd])
            nc.sync.dma_start(out=w_t[:, o:o + wd], in_=w_d[:, o:o + wd])
            o += wd
        xa_t = const_pool.tile([2 * D, B], bf16, name="xa_t")
        nc.sync.dma_start(out=xa_t[:], in_=xa_d[:])
        bias_t = const_pool.tile([BT, NBT], f32, name="bias_t")
        nc.sync.dma_start(out=bias_t[:], in_=bias_d[:])
        eps_t = const_pool.tile([BT, 1], f32, name="eps_t")
        nc.vector.memset(eps_t[:], float(EPS))

        res = res_pool.tile([BT, NBT * NUM_CLASSES], f32, name="res")

        bounces = []
        for t in range(NBT):
            bi = dram_pool.tile([BT, NUM_CLASSES], f32, name=f"bnc_in{t}")
            bo = dram_pool.tile(
                [BT, NUM_CLASSES], f32, name=f"bnc_out{t}", addr_space="Shared"
            )
            bounces.append((bi, bo))

        for t in range(NBT):
            half = (t % 2) * D
            sims = sims_pool.tile([BT, e_pad], bf16, tag="sims")
            lhsT = xa_t[half:half + D, t * BT:(t + 1) * BT]
            for (wo, wl) in wins:
                ps = psum_pool.tile([BT, WIN], f32, tag="ps")
                for co in range(0, wl, CHUNK):
                    cl = min(CHUNK, wl - co)
                    nc.tensor.matmul(
                        ps[:, co:co + cl],
                        lhsT=lhsT,
                        rhs=ea_t[half:half + D, wo + co:wo + co + cl],
                        start=True,
                        stop=True,
                    )
                nc.scalar.activation(
                    sims[:, wo:wo + wl],
                    ps[:, :wl],
                    mybir.ActivationFunctionType.Exp,
                    bias=bias_t[:, t:t + 1],
                    scale=1.0,
                )
            cls = cls_pool.tile([BT, NUM_CLASSES], f32, tag="cls")
            junk = junk_pool.tile([BT, max_seg], bf16, tag="junk")
            for c in range(NUM_CLASSES):
                o = int(seg_offs[c])
                s = seg_sizes[c]
                if c >= NUM_CLASSES - N_GPSIMD_CLASSES:
                    nc.gpsimd.scalar_tensor_tensor(
                        junk[:, :s],
                        sims[:, o:o + s],
                        1.0,
                        w_t[:, o:o + s],
                        mybir.AluOpType.mult,
                        mybir.AluOpType.mult,
                        accum_out=cls[:, c:c + 1],
                    )
                else:
                    nc.vector.affine_mul_reduce(
                        junk[:, :s],
                        cls[:, c:c + 1],
                        sims[:, o:o + s],
                        w_t[:, o:o + s],
                        1.0,
                        0.0,
                    )
            bi, bo = bounces[t]
            nc.sync.dma_start(out=bi[:], in_=cls[:])
            nc.gpsimd.collective_compute(
                "AllReduce",
                mybir.AluOpType.add,
                replica_groups=[list(range(N_CORES))],
                ins=[bi[:].opt()],
                outs=[bo[:].opt()],
            )
            nc.sync.dma_start(
                out=res[:, t * NUM_CLASSES:(t + 1) * NUM_CLASSES], in_=bo[:]
            )

        logit = res_pool.tile([BT, NBT * NUM_CLASSES], f32, name="logit")
        nc.scalar.activation(
            logit[:],
            res[:],
            mybir.ActivationFunctionType.Ln,
            bias=eps_t[:, 0:1],
            scale=float(np.exp(CSHIFT)),
        )
        out_ap = out_d.rearrange("(t p) c -> p t c", p=BT)
        nc.sync.dma_start(out=out_ap, in_=logit[:].rearrange("p (t c) -> p t c", t=NBT))

    nc.compile()
    return nc


def kernel(x, exemplars, exemplar_labels, beta_raw):
    global LAST_EXEC_NS, LAST_RESULTS
    from concourse.bass_utils import run_bass_kernel_spmd

    ea_cores, w_cores, xa, bias, seg_offs, seg_sizes, e_pad = _host_prep(
        x, exemplars, exemplar_labels, beta_raw
    )
    nc = _build_program(seg_offs, seg_sizes, e_pad)

    in_maps = [
        {"ea": ea_cores[i], "w": w_cores[i], "xa": xa, "biasx": bias}
        for i in range(N_CORES)
    ]
    kwargs = {}
    if TRACE:
        kwargs["trace"] = True
        if TRACE_DIR:
            os.makedirs(TRACE_DIR, exist_ok=True)
            kwargs["tmpdir"] = TRACE_DIR
    ret = run_bass_kernel_spmd(nc, in_maps, list(range(N_CORES)), **kwargs)
    LAST_EXEC_NS = ret.exec_time_ns
    LAST_RESULTS = ret
    out = np.asarray(ret.results[0]["logits"], dtype=np.float32)
    return out
